# revision 19
# baseline (speedup 1.0000x reference)
# Multi-head attention kernel for 8 TRN2 NeuronCores.
#
# Sharding: data-parallel over batch. B=16 batches -> 2 per core; weights
# replicated; no collectives. Each core runs the full attention stack on
# its 2 batches.
#
# v2 design (bf16 compute, fp32 accumulate):
#   - inputs cast f32->bf16 during the HBM->SBUF DMA itself (SWDGE cast)
#   - q,k,v,d,W transposed by batched HWDGE transpose-DMAs (no PE work)
#   - qh^T, kh^T = W^T.T @ q^T; vh natural = v^T.T @ Wv^T   (bf16 matmuls)
#   - scores^T[m,n] = kh^T.T @ qh^T per head; head PAIRS packed into the
#     PE array (rows 0-63 / 64-127), their softmax stats and att@v
#     col-packed via tile_position (0,0)/(0,64) into shared PSUM banks
#   - softmax uses exp(s+d) = exp(s)*exp(d): g=exp(d^T), f=d*exp(d) are
#     precomputed per batch, so both DVE passes run at 2x bf16 mode:
#       TT1 = e*g feeds the ones-matmul denominator, TT2 = e*f feeds att@v
#   - 1/sums via ScalarE ln then exp(-x) (DVE iterative divide is ~8x slower)
#   - x^T normalized on PSUM evacuation; out = x^T.T @ Wp^T lands natural
#   - biases are all-zero per the problem spec; accepted but not added
import os
import numpy as np

B, N, E, H = 16, 1024, 512, 8
DH = E // H
NCORES = 8
BL = B // NCORES  # batches per core
P = 128
NT = N // P  # 8 seq tiles
ET = E // P  # 4 embed tiles
NC2 = N // 512  # 2 n-chunks of 512
HP = H // 2  # 4 head pairs

_graph_cache = {}


def build_graph():
    import concourse.bacc as bacc
    import concourse.tile as tile
    import concourse.mybir as mybir
    from contextlib import ExitStack
    from concourse.masks import make_identity

    dt = mybir.dt
    f32 = dt.float32
    bf16 = dt.bfloat16
    AF = mybir.ActivationFunctionType

    nc = bacc.Bacc(
        "TRN2", target_bir_lowering=False, debug=False, num_devices=NCORES
    )

    q_d = nc.dram_tensor("q", [BL, N, E], f32, kind="ExternalInput").ap()
    k_d = nc.dram_tensor("k", [BL, N, E], f32, kind="ExternalInput").ap()
    v_d = nc.dram_tensor("v", [BL, N, E], f32, kind="ExternalInput").ap()
    d_d = nc.dram_tensor("d", [BL, N, N], f32, kind="ExternalInput").ap()
    w_d = {
        w: nc.dram_tensor(w, [E, E], f32, kind="ExternalInput").ap()
        for w in ("Wq", "Wk", "Wv", "Wp")
    }
    for bname in ("bq", "bk", "bv", "bp"):
        # all-zero per the problem spec; declared so the harness can bind them
        nc.dram_tensor(bname, [E], f32, kind="ExternalInput")
    out_d = nc.dram_tensor("out", [BL, N, E], f32, kind="ExternalOutput").ap()

    with tile.TileContext(nc) as tc, ExitStack() as ctx:
        wpool = ctx.enter_context(tc.tile_pool(name="wts", bufs=1))
        natp = ctx.enter_context(tc.tile_pool(name="nat", bufs=4))
        actp = ctx.enter_context(tc.tile_pool(name="acts", bufs=1))
        smp = ctx.enter_context(tc.tile_pool(name="softmax", bufs=3))
        outp = ctx.enter_context(tc.tile_pool(name="outs", bufs=3))
        psp = ctx.enter_context(tc.tile_pool(name="ps", bufs=2, space="PSUM"))

        ones64 = wpool.tile([P, 64], bf16)
        nc.gpsimd.memset(ones64[:], 1.0)
        ident = wpool.tile([P, P], bf16)
        make_identity(nc, ident[:])

        # ---- weights: SWDGE cast-load to SBUF, transpose on TensorE ----
        wT = {}
        for name in ("Wq", "Wk", "Wv", "Wp"):
            tiles = []
            for et in range(ET):
                tiles.append(
                    wpool.tile(
                        [P, E], bf16, tag=f"wT_{name}_{et}", name=f"wT_{name}_{et}"
                    )
                )
            for ot in range(ET):
                wnat = natp.tile([P, E], bf16, tag="nat", name=f"wnat_{name}_{ot}")
                nc.gpsimd.dma_start(wnat[:], w_d[name][ot * P : (ot + 1) * P, :])
                pst = psp.tile([P, 512], bf16, tag="ps_t", bufs=2,
                               padded_shape=[P, 1024], name=f"pswt_{name}_{ot}")
                for et in range(ET):
                    nc.tensor.transpose(
                        pst[:, et * P : (et + 1) * P],
                        wnat[:, et * P : (et + 1) * P],
                        ident[:],
                    )
                for et in range(ET):
                    nc.vector.tensor_copy(
                        tiles[et][:, ot * P : (ot + 1) * P],
                        pst[:, et * P : (et + 1) * P],
                    )
            wT[name] = tiles

        def make_stage_a(b):
            """Allocate batch-b transposed input tensors and return
            (bigs, thunks): each thunk emits one nt-block's cast-load,
            TensorE transposes and scatter-copy. Thunks are drained
            interleaved into the previous batch's attention emission so
            their DMA/DVE work overlaps it (engine queues are in-order).
            dT alternates buffers by parity so batch b+1's d can load
            while batch b's g/f are still being consumed."""
            specs = (
                ("vT", v_d, ET),
                ("qT", q_d, ET),
                ("kT", k_d, ET),
                ("dT", d_d, NT),
            )
            bigs = {}
            for tag, _, ets in specs:
                slot_tag = f"dT_all{b % 2}" if tag == "dT" else f"{tag}_all"
                bigs[tag] = actp.tile(
                    [P, ets * N], bf16, tag=slot_tag, name=f"t_{tag}_{b}"
                )
            thunks = []
            for tag, x_dram, ets in specs:
                big = bigs[tag]
                for nt in range(NT):
                    def chunk(tag=tag, x_dram=x_dram, ets=ets, big=big, nt=nt):
                        ecols = ets * P
                        xnat = natp.tile(
                            [P, ecols], bf16, tag="nat",
                            name=f"nat_{tag}_{b}_{nt}",
                        )
                        nc.gpsimd.dma_start(
                            xnat[:], x_dram[b, nt * P : (nt + 1) * P, :]
                        )
                        for eg in range(ets // 4):
                            pst = psp.tile(
                                [P, 512], bf16, tag="ps_t", bufs=2,
                                padded_shape=[P, 1024],
                                name=f"pst_{tag}_{b}_{nt}_{eg}",
                            )
                            for j in range(4):
                                et = eg * 4 + j
                                nc.tensor.transpose(
                                    pst[:, j * P : (j + 1) * P],
                                    xnat[:, et * P : (et + 1) * P],
                                    ident[:],
                                )
                            dst = big[:].rearrange(
                                "p (t n) -> p t n", t=ets
                            )[:, eg * 4 : eg * 4 + 4, nt * P : (nt + 1) * P]
                            nc.vector.tensor_copy(
                                dst, pst[:].rearrange("p (t n) -> p t n", t=4)
                            )
                    thunks.append(chunk)
            return bigs, thunks

        def phase_a(b, bigs):
            """g/f precompute + q/k/v projections for batch b."""
            dT_all = bigs["dT"]
            vT = [bigs["vT"][:, et * N : (et + 1) * N] for et in range(ET)]
            qT = [bigs["qT"][:, et * N : (et + 1) * N] for et in range(ET)]
            kT = [bigs["kT"][:, et * N : (et + 1) * N] for et in range(ET)]

            g_all = actp.tile([P, NT * N], bf16, tag="g_all", name=f"g_all{b}")
            f_all = actp.tile([P, NT * N], bf16, tag="f_all", name=f"f_all{b}")
            for mt in range(NT):
                dsl = slice(mt * N, (mt + 1) * N)
                nc.scalar.activation(g_all[:, dsl], dT_all[:, dsl], AF.Exp)
                nc.vector.tensor_mul(
                    f_all[:, dsl], g_all[:, dsl], dT_all[:, dsl]
                )
            gT = [g_all[:, mt * N : (mt + 1) * N] for mt in range(NT)]
            fT = [f_all[:, mt * N : (mt + 1) * N] for mt in range(NT)]

            hT = {}
            for xname, xT, wname in (("q", qT, "Wq"), ("k", kT, "Wk")):
                tiles = []
                for ot in range(ET):
                    tiles.append(
                        actp.tile(
                            [P, N], bf16,
                            tag=f"hT_{xname}_{ot}",
                            name=f"hT_{xname}_{ot}_{b}",
                        )
                    )
                for ot in range(ET):
                    ps = psp.tile([P, 1024], f32, tag="ps_pair", bufs=2)
                    for nch in range(NC2):
                        for et in range(ET):
                            nc.tensor.matmul(
                                ps[:, nch * 512 : (nch + 1) * 512],
                                wT[wname][et][:, ot * P : (ot + 1) * P],
                                xT[et][:, nch * 512 : (nch + 1) * 512],
                                start=(et == 0),
                                stop=(et == ET - 1),
                            )
                    if xname == "q":
                        nc.vector.tensor_scalar_mul(
                            tiles[ot][:], ps[:], 1.0 / (DH**0.5)
                        )
                    else:
                        nc.vector.tensor_copy(tiles[ot][:], ps[:])
                hT[xname] = tiles

            vh_all = actp.tile(
                [P, NT * E], bf16, tag="vh_all", name=f"vh_all{b}"
            )
            for mtp in range(NT // 2):
                ps = psp.tile([P, 1024], f32, tag="ps_pair", bufs=2)
                for j in range(2):
                    mt = 2 * mtp + j
                    for et in range(ET):
                        nc.tensor.matmul(
                            ps[:, j * 512 : (j + 1) * 512],
                            vT[et][:, mt * P : (mt + 1) * P],
                            wT["Wv"][et][:, :],
                            start=(et == 0),
                            stop=(et == ET - 1),
                        )
                nc.vector.tensor_copy(
                    vh_all[:, mtp * 1024 : (mtp + 1) * 1024], ps[:]
                )
            return gT, fT, hT, vh_all

        def phase_c(b, gT, fT, hT, vh_all, pending):
            """attention for batch b; drains next batch's load thunks."""
            sums_all = actp.tile(
                [P, H * 512], bf16, tag="sums_all", name=f"sums_all{b}"
            )
            xu_all = actp.tile(
                [P, H * 512], bf16, tag="xu_all", name=f"xu_all{b}"
            )
            for hp in range(HP):
                h0, h1 = 2 * hp, 2 * hp + 1
                for ncc in range(NC2):
                    nsl = slice(ncc * 512, (ncc + 1) * 512)
                    slot = hp * 2 + ncc
                    ps_sum = psp.tile([P, 512], f32, tag="ps_sum", bufs=1)
                    ps_x = psp.tile([P, 512], f32, tag="ps_x", bufs=1)

                    def emit_scores(mt):
                        msl = slice(mt * P, (mt + 1) * P)
                        pp = psp.tile(
                            [P, 1024], f32, tag="ps_pair", bufs=2,
                            name=f"pp_{b}_{hp}_{ncc}_{mt}",
                        )
                        nc.tensor.matmul(
                            pp[:, 0:512],
                            hT["k"][hp][0:64, msl],
                            hT["q"][hp][0:64, nsl],
                            start=True, stop=True,
                        )
                        nc.tensor.matmul(
                            pp[:, 512:1024],
                            hT["k"][hp][64:128, msl],
                            hT["q"][hp][64:128, nsl],
                            start=True, stop=True,
                        )
                        return pp

                    pps = [emit_scores(0), emit_scores(1)]
                    for mt in range(NT):
                        pp = pps.pop(0)
                        e01 = smp.tile([P, 1024], bf16, tag="e01")
                        nc.scalar.activation(e01[:], pp[:], AF.Exp)
                        if mt + 2 < NT:
                            pps.append(emit_scores(mt + 2))
                        gb = (
                            gT[mt][:, nsl]
                            .rearrange("p (o f) -> p o f", o=1)
                            .broadcast_to((P, 2, 512))
                        )
                        fb = (
                            fT[mt][:, nsl]
                            .rearrange("p (o f) -> p o f", o=1)
                            .broadcast_to((P, 2, 512))
                        )
                        e2 = e01[:].rearrange("p (o f) -> p o f", o=2)
                        t1 = smp.tile([P, 1024], bf16, tag="t1")
                        nc.vector.tensor_mul(
                            t1[:].rearrange("p (o f) -> p o f", o=2), e2, gb
                        )
                        t2 = smp.tile([P, 1024], bf16, tag="t2")
                        nc.vector.tensor_mul(
                            t2[:].rearrange("p (o f) -> p o f", o=2), e2, fb
                        )
                        nc.tensor.matmul(
                            ps_sum[0:64, :], ones64[:], t1[:, 0:512],
                            start=(mt == 0), stop=(mt == NT - 1),
                            skip_group_check=True,
                        )
                        nc.tensor.matmul(
                            ps_sum[64:128, :], ones64[:], t1[:, 512:1024],
                            start=(mt == 0), stop=(mt == NT - 1),
                            skip_group_check=True, tile_position=(0, 64),
                        )
                        nc.tensor.matmul(
                            ps_x[0:64, :],
                            vh_all[:, mt * 512 + h0 * 64 : mt * 512 + h0 * 64 + 64],
                            t2[:, 0:512],
                            start=(mt == 0), stop=(mt == NT - 1),
                            skip_group_check=True,
                        )
                        nc.tensor.matmul(
                            ps_x[64:128, :],
                            vh_all[:, mt * 512 + h1 * 64 : mt * 512 + h1 * 64 + 64],
                            t2[:, 512:1024],
                            start=(mt == 0), stop=(mt == NT - 1),
                            skip_group_check=True, tile_position=(0, 64),
                        )
                    nc.vector.tensor_copy(
                        sums_all[:, slot * 512 : (slot + 1) * 512], ps_sum[:]
                    )
                    nc.vector.tensor_copy(
                        xu_all[:, slot * 512 : (slot + 1) * 512], ps_x[:]
                    )
                    for th in pending[:4]:
                        th()
                    pending = pending[4:]
            for th in pending:
                th()
            return sums_all, xu_all

        def phase_t(b, sums_all, xu_all):
            """batched reciprocal, normalize, output projection for b."""
            x_all = actp.tile([P, HP * N], bf16, tag="x_all", name=f"x_all{b}")
            lnt = smp.tile([P, H * 512], f32, tag="lnt", bufs=1)
            nc.scalar.activation(lnt[:], sums_all[:], AF.Ln)
            nc.scalar.activation(lnt[:], lnt[:], AF.Exp, scale=-1.0)
            nc.vector.tensor_mul(x_all[:], xu_all[:], lnt[:])

            for ntp in range(NT // 2):
                ps = psp.tile([P, 1024], f32, tag="ps_pair", bufs=2)
                for j in range(2):
                    nt = 2 * ntp + j
                    for hp in range(HP):
                        nc.tensor.matmul(
                            ps[:, j * 512 : (j + 1) * 512],
                            x_all[:, hp * N + nt * P : hp * N + (nt + 1) * P],
                            wT["Wp"][hp][:, :],
                            start=(hp == 0),
                            stop=(hp == HP - 1),
                        )
                ot_sb = outp.tile([P, 1024], f32, tag="ot_sb", bufs=2)
                nc.vector.tensor_copy(ot_sb[:], ps[:])
                nc.sync.dma_start(
                    out_d[
                        b, ntp * 2 * P : (ntp + 1) * 2 * P, :
                    ].rearrange("(c p) e -> p c e", p=P),
                    ot_sb[:].rearrange("p (c e) -> p c e", c=2),
                )

        # schedule: A0 C0(+loads1) A1 T0 C1 T1 so the batch transition
        # keeps every engine fed (next batch's prep overlaps the tail)
        bigs0, thunks0 = make_stage_a(0)
        for th in thunks0:
            th()
        a0 = phase_a(0, bigs0)
        bigs1, thunks1 = make_stage_a(1)
        c0 = phase_c(0, *a0, pending=thunks1)
        a1 = phase_a(1, bigs1)
        phase_t(0, *c0)
        c1 = phase_c(1, *a1, pending=[])
        phase_t(1, *c1)

    nc.compile()
    return nc


def _get_graph():
    if "nc" not in _graph_cache:
        _graph_cache["nc"] = build_graph()
    return _graph_cache["nc"]


def make_in_maps(full):
    in_maps = []
    for c in range(NCORES):
        bsl = slice(c * BL, (c + 1) * BL)
        m = {
            "q": full["q"][bsl],
            "k": full["k"][bsl],
            "v": full["v"][bsl],
            "d": full["d"][bsl],
        }
        for w in ("Wq", "Wk", "Wv", "Wp", "bq", "bk", "bv", "bp"):
            m[w] = full[w]
        in_maps.append(m)
    return in_maps


def kernel(**inputs):
    from concourse.bass_utils import run_bass_kernel_spmd

    nc = _get_graph()
    full = {
        k: np.ascontiguousarray(np.asarray(v, np.float32))
        for k, v in inputs.items()
    }
    res = run_bass_kernel_spmd(
        nc,
        make_in_maps(full),
        core_ids=list(range(NCORES)),
        trace=bool(os.environ.get("ATTN_TRACE")),
    )
    if res.exec_time_ns is not None:
        _graph_cache["exec_time_ns"] = res.exec_time_ns
        _graph_cache["profile_json"] = res.profile_json
        _graph_cache["trace"] = res.instructions_and_trace
    out = np.concatenate([res.results[c]["out"] for c in range(NCORES)], axis=0)
    return out


# revision 20
# speedup vs baseline: 1.0183x; 1.0183x over previous
# Multi-head attention kernel for 8 TRN2 NeuronCores.
#
# Sharding: data-parallel over batch. B=16 batches -> 2 per core; weights
# replicated; no collectives. Each core runs the full attention stack on
# its 2 batches.
#
# v2 design (bf16 compute, fp32 accumulate):
#   - inputs cast f32->bf16 during the HBM->SBUF DMA itself (SWDGE cast)
#   - q,k,v,d,W transposed by batched HWDGE transpose-DMAs (no PE work)
#   - qh^T, kh^T = W^T.T @ q^T; vh natural = v^T.T @ Wv^T   (bf16 matmuls)
#   - scores^T[m,n] = kh^T.T @ qh^T per head; head PAIRS packed into the
#     PE array (rows 0-63 / 64-127), their softmax stats and att@v
#     col-packed via tile_position (0,0)/(0,64) into shared PSUM banks
#   - softmax uses exp(s+d) = exp(s)*exp(d): g=exp(d^T), f=d*exp(d) are
#     precomputed per batch, so both DVE passes run at 2x bf16 mode:
#       TT1 = e*g feeds the ones-matmul denominator, TT2 = e*f feeds att@v
#   - 1/sums via ScalarE ln then exp(-x) (DVE iterative divide is ~8x slower)
#   - x^T normalized on PSUM evacuation; out = x^T.T @ Wp^T lands natural
#   - biases are all-zero per the problem spec; accepted but not added
import os
import numpy as np

B, N, E, H = 16, 1024, 512, 8
DH = E // H
NCORES = 8
BL = B // NCORES  # batches per core
P = 128
NT = N // P  # 8 seq tiles
ET = E // P  # 4 embed tiles
NC2 = N // 512  # 2 n-chunks of 512
HP = H // 2  # 4 head pairs

_graph_cache = {}


def build_graph():
    import concourse.bacc as bacc
    import concourse.tile as tile
    import concourse.mybir as mybir
    from contextlib import ExitStack
    from concourse.masks import make_identity

    dt = mybir.dt
    f32 = dt.float32
    bf16 = dt.bfloat16
    AF = mybir.ActivationFunctionType

    nc = bacc.Bacc(
        "TRN2", target_bir_lowering=False, debug=False, num_devices=NCORES
    )

    q_d = nc.dram_tensor("q", [BL, N, E], f32, kind="ExternalInput").ap()
    k_d = nc.dram_tensor("k", [BL, N, E], f32, kind="ExternalInput").ap()
    v_d = nc.dram_tensor("v", [BL, N, E], f32, kind="ExternalInput").ap()
    d_d = nc.dram_tensor("d", [BL, N, N], f32, kind="ExternalInput").ap()
    w_d = {
        w: nc.dram_tensor(w, [E, E], f32, kind="ExternalInput").ap()
        for w in ("Wq", "Wk", "Wv", "Wp")
    }
    for bname in ("bq", "bk", "bv", "bp"):
        # all-zero per the problem spec; declared so the harness can bind them
        nc.dram_tensor(bname, [E], f32, kind="ExternalInput")
    out_d = nc.dram_tensor("out", [BL, N, E], f32, kind="ExternalOutput").ap()

    with tile.TileContext(nc) as tc, ExitStack() as ctx:
        wpool = ctx.enter_context(tc.tile_pool(name="wts", bufs=1))
        natp = ctx.enter_context(tc.tile_pool(name="nat", bufs=4))
        actp = ctx.enter_context(tc.tile_pool(name="acts", bufs=1))
        smp = ctx.enter_context(tc.tile_pool(name="softmax", bufs=3))
        outp = ctx.enter_context(tc.tile_pool(name="outs", bufs=3))
        psp = ctx.enter_context(tc.tile_pool(name="ps", bufs=2, space="PSUM"))

        ones64 = wpool.tile([P, 64], bf16)
        nc.gpsimd.memset(ones64[:], 1.0)
        ident = wpool.tile([P, P], bf16)
        make_identity(nc, ident[:])

        # ---- weights: SWDGE cast-load to SBUF, transpose on TensorE ----
        wT = {}
        for name in ("Wq", "Wk", "Wv", "Wp"):
            tiles = []
            for et in range(ET):
                tiles.append(
                    wpool.tile(
                        [P, E], bf16, tag=f"wT_{name}_{et}", name=f"wT_{name}_{et}"
                    )
                )
            for ot in range(ET):
                wnat = natp.tile([P, E], bf16, tag="nat", name=f"wnat_{name}_{ot}")
                nc.gpsimd.dma_start(wnat[:], w_d[name][ot * P : (ot + 1) * P, :])
                pst = psp.tile([P, 512], bf16, tag="ps_t", bufs=2,
                               padded_shape=[P, 1024], name=f"pswt_{name}_{ot}")
                for et in range(ET):
                    nc.tensor.transpose(
                        pst[:, et * P : (et + 1) * P],
                        wnat[:, et * P : (et + 1) * P],
                        ident[:],
                    )
                for et in range(ET):
                    nc.vector.tensor_copy(
                        tiles[et][:, ot * P : (ot + 1) * P],
                        pst[:, et * P : (et + 1) * P],
                    )
            wT[name] = tiles

        def make_stage_a(b):
            """Allocate batch-b transposed input tensors and return
            (bigs, thunks): each thunk emits one nt-block's cast-load,
            TensorE transposes and scatter-copy. Thunks are drained
            interleaved into the previous batch's attention emission so
            their DMA/DVE work overlaps it (engine queues are in-order).
            dT alternates buffers by parity so batch b+1's d can load
            while batch b's g/f are still being consumed."""
            specs = (
                ("vT", v_d, ET),
                ("qT", q_d, ET),
                ("kT", k_d, ET),
                ("dT", d_d, NT),
            )
            bigs = {}
            for tag, _, ets in specs:
                slot_tag = f"dT_all{b % 2}" if tag == "dT" else f"{tag}_all"
                bigs[tag] = actp.tile(
                    [P, ets * N], bf16, tag=slot_tag, name=f"t_{tag}_{b}"
                )
            thunks = []
            for tag, x_dram, ets in specs:
                big = bigs[tag]
                for nt in range(NT):
                    def chunk(tag=tag, x_dram=x_dram, ets=ets, big=big, nt=nt):
                        ecols = ets * P
                        xnat = natp.tile(
                            [P, ecols], bf16, tag="nat",
                            name=f"nat_{tag}_{b}_{nt}",
                        )
                        nc.gpsimd.dma_start(
                            xnat[:], x_dram[b, nt * P : (nt + 1) * P, :]
                        )
                        for eg in range(ets // 4):
                            pst = psp.tile(
                                [P, 512], bf16, tag="ps_t", bufs=2,
                                padded_shape=[P, 1024],
                                name=f"pst_{tag}_{b}_{nt}_{eg}",
                            )
                            for j in range(4):
                                et = eg * 4 + j
                                nc.tensor.transpose(
                                    pst[:, j * P : (j + 1) * P],
                                    xnat[:, et * P : (et + 1) * P],
                                    ident[:],
                                )
                            dst = big[:].rearrange(
                                "p (t n) -> p t n", t=ets
                            )[:, eg * 4 : eg * 4 + 4, nt * P : (nt + 1) * P]
                            nc.vector.tensor_copy(
                                dst, pst[:].rearrange("p (t n) -> p t n", t=4)
                            )
                    thunks.append(chunk)
            return bigs, thunks

        def phase_gf(b, bigs):
            """g = exp(d^T), f = d^T*g for batch b."""
            dT_all = bigs["dT"]
            g_all = actp.tile([P, NT * N], bf16, tag="g_all", name=f"g_all{b}")
            f_all = actp.tile([P, NT * N], bf16, tag="f_all", name=f"f_all{b}")
            for mt in range(NT):
                dsl = slice(mt * N, (mt + 1) * N)
                nc.scalar.activation(g_all[:, dsl], dT_all[:, dsl], AF.Exp)
                nc.vector.tensor_mul(
                    f_all[:, dsl], g_all[:, dsl], dT_all[:, dsl]
                )
            gT = [g_all[:, mt * N : (mt + 1) * N] for mt in range(NT)]
            fT = [f_all[:, mt * N : (mt + 1) * N] for mt in range(NT)]
            return gT, fT

        def phase_a(b, bigs):
            """q/k/v projections for batch b."""
            vT = [bigs["vT"][:, et * N : (et + 1) * N] for et in range(ET)]
            qT = [bigs["qT"][:, et * N : (et + 1) * N] for et in range(ET)]
            kT = [bigs["kT"][:, et * N : (et + 1) * N] for et in range(ET)]

            hT = {}
            for xname, xT, wname in (("q", qT, "Wq"), ("k", kT, "Wk")):
                tiles = []
                for ot in range(ET):
                    tiles.append(
                        actp.tile(
                            [P, N], bf16,
                            tag=f"hT_{xname}_{ot}",
                            name=f"hT_{xname}_{ot}_{b}",
                        )
                    )
                for ot in range(ET):
                    ps = psp.tile([P, 1024], f32, tag="ps_pair", bufs=2)
                    for nch in range(NC2):
                        for et in range(ET):
                            nc.tensor.matmul(
                                ps[:, nch * 512 : (nch + 1) * 512],
                                wT[wname][et][:, ot * P : (ot + 1) * P],
                                xT[et][:, nch * 512 : (nch + 1) * 512],
                                start=(et == 0),
                                stop=(et == ET - 1),
                            )
                    if xname == "q":
                        nc.vector.tensor_scalar_mul(
                            tiles[ot][:], ps[:], 1.0 / (DH**0.5)
                        )
                    else:
                        nc.vector.tensor_copy(tiles[ot][:], ps[:])
                hT[xname] = tiles

            vh_all = actp.tile(
                [P, NT * E], bf16, tag="vh_all", name=f"vh_all{b}"
            )
            for mtp in range(NT // 2):
                ps = psp.tile([P, 1024], f32, tag="ps_pair", bufs=2)
                for j in range(2):
                    mt = 2 * mtp + j
                    for et in range(ET):
                        nc.tensor.matmul(
                            ps[:, j * 512 : (j + 1) * 512],
                            vT[et][:, mt * P : (mt + 1) * P],
                            wT["Wv"][et][:, :],
                            start=(et == 0),
                            stop=(et == ET - 1),
                        )
                nc.vector.tensor_copy(
                    vh_all[:, mtp * 1024 : (mtp + 1) * 1024], ps[:]
                )
            return hT, vh_all

        def phase_c(b, gT, fT, hT, vh_all, pending):  # noqa: D401
            """attention for batch b; drains next batch's load thunks."""
            sums_all = actp.tile(
                [P, H * 512], bf16, tag="sums_all", name=f"sums_all{b}"
            )
            xu_all = actp.tile(
                [P, H * 512], bf16, tag="xu_all", name=f"xu_all{b}"
            )
            for hp in range(HP):
                h0, h1 = 2 * hp, 2 * hp + 1
                for ncc in range(NC2):
                    nsl = slice(ncc * 512, (ncc + 1) * 512)
                    slot = hp * 2 + ncc
                    ps_sum = psp.tile([P, 512], f32, tag="ps_sum", bufs=1)
                    ps_x = psp.tile([P, 512], f32, tag="ps_x", bufs=1)

                    def emit_scores(mt):
                        msl = slice(mt * P, (mt + 1) * P)
                        pp = psp.tile(
                            [P, 1024], f32, tag="ps_pair", bufs=2,
                            name=f"pp_{b}_{hp}_{ncc}_{mt}",
                        )
                        nc.tensor.matmul(
                            pp[:, 0:512],
                            hT["k"][hp][0:64, msl],
                            hT["q"][hp][0:64, nsl],
                            start=True, stop=True,
                        )
                        nc.tensor.matmul(
                            pp[:, 512:1024],
                            hT["k"][hp][64:128, msl],
                            hT["q"][hp][64:128, nsl],
                            start=True, stop=True,
                        )
                        return pp

                    pps = [emit_scores(0), emit_scores(1)]
                    for mt in range(NT):
                        pp = pps.pop(0)
                        e01 = smp.tile([P, 1024], bf16, tag="e01")
                        nc.scalar.activation(e01[:], pp[:], AF.Exp)
                        if mt + 2 < NT:
                            pps.append(emit_scores(mt + 2))
                        gb = (
                            gT[mt][:, nsl]
                            .rearrange("p (o f) -> p o f", o=1)
                            .broadcast_to((P, 2, 512))
                        )
                        fb = (
                            fT[mt][:, nsl]
                            .rearrange("p (o f) -> p o f", o=1)
                            .broadcast_to((P, 2, 512))
                        )
                        e2 = e01[:].rearrange("p (o f) -> p o f", o=2)
                        t1 = smp.tile([P, 1024], bf16, tag="t1")
                        nc.vector.tensor_mul(
                            t1[:].rearrange("p (o f) -> p o f", o=2), e2, gb
                        )
                        t2 = smp.tile([P, 1024], bf16, tag="t2")
                        nc.vector.tensor_mul(
                            t2[:].rearrange("p (o f) -> p o f", o=2), e2, fb
                        )
                        nc.tensor.matmul(
                            ps_sum[0:64, :], ones64[:], t1[:, 0:512],
                            start=(mt == 0), stop=(mt == NT - 1),
                            skip_group_check=True,
                        )
                        nc.tensor.matmul(
                            ps_sum[64:128, :], ones64[:], t1[:, 512:1024],
                            start=(mt == 0), stop=(mt == NT - 1),
                            skip_group_check=True, tile_position=(0, 64),
                        )
                        nc.tensor.matmul(
                            ps_x[0:64, :],
                            vh_all[:, mt * 512 + h0 * 64 : mt * 512 + h0 * 64 + 64],
                            t2[:, 0:512],
                            start=(mt == 0), stop=(mt == NT - 1),
                            skip_group_check=True,
                        )
                        nc.tensor.matmul(
                            ps_x[64:128, :],
                            vh_all[:, mt * 512 + h1 * 64 : mt * 512 + h1 * 64 + 64],
                            t2[:, 512:1024],
                            start=(mt == 0), stop=(mt == NT - 1),
                            skip_group_check=True, tile_position=(0, 64),
                        )
                    nc.vector.tensor_copy(
                        sums_all[:, slot * 512 : (slot + 1) * 512], ps_sum[:]
                    )
                    nc.vector.tensor_copy(
                        xu_all[:, slot * 512 : (slot + 1) * 512], ps_x[:]
                    )
                    for th in pending[:4]:
                        th()
                    pending = pending[4:]
            for th in pending:
                th()
            return sums_all, xu_all

        def phase_t(b, sums_all, xu_all):
            """batched reciprocal, normalize, output projection for b."""
            x_all = actp.tile([P, HP * N], bf16, tag="x_all", name=f"x_all{b}")
            lnt = smp.tile([P, H * 512], f32, tag="lnt", bufs=1)
            nc.scalar.activation(lnt[:], sums_all[:], AF.Ln)
            nc.scalar.activation(lnt[:], lnt[:], AF.Exp, scale=-1.0)
            nc.vector.tensor_mul(x_all[:], xu_all[:], lnt[:])

            for ntp in range(NT // 2):
                ps = psp.tile([P, 1024], f32, tag="ps_pair", bufs=2)
                for j in range(2):
                    nt = 2 * ntp + j
                    for hp in range(HP):
                        nc.tensor.matmul(
                            ps[:, j * 512 : (j + 1) * 512],
                            x_all[:, hp * N + nt * P : hp * N + (nt + 1) * P],
                            wT["Wp"][hp][:, :],
                            start=(hp == 0),
                            stop=(hp == HP - 1),
                        )
                ot_sb = outp.tile([P, 1024], f32, tag="ot_sb", bufs=2)
                nc.vector.tensor_copy(ot_sb[:], ps[:])
                nc.sync.dma_start(
                    out_d[
                        b, ntp * 2 * P : (ntp + 1) * 2 * P, :
                    ].rearrange("(c p) e -> p c e", p=P),
                    ot_sb[:].rearrange("p (c e) -> p c e", c=2),
                )

        # schedule: A0 C0(+loads1) A1 T0 C1 T1 so the batch transition
        # keeps every engine fed (next batch's prep overlaps the tail)
        bigs0, thunks0 = make_stage_a(0)
        for th in thunks0:
            th()
        gf0 = phase_gf(0, bigs0)
        a0 = phase_a(0, bigs0)
        bigs1, thunks1 = make_stage_a(1)
        c0 = phase_c(0, *gf0, *a0, pending=thunks1)
        gf1 = phase_gf(1, bigs1)
        phase_t(0, *c0)
        a1 = phase_a(1, bigs1)
        c1 = phase_c(1, *gf1, *a1, pending=[])
        phase_t(1, *c1)

    nc.compile()
    return nc


def _get_graph():
    if "nc" not in _graph_cache:
        _graph_cache["nc"] = build_graph()
    return _graph_cache["nc"]


def make_in_maps(full):
    in_maps = []
    for c in range(NCORES):
        bsl = slice(c * BL, (c + 1) * BL)
        m = {
            "q": full["q"][bsl],
            "k": full["k"][bsl],
            "v": full["v"][bsl],
            "d": full["d"][bsl],
        }
        for w in ("Wq", "Wk", "Wv", "Wp", "bq", "bk", "bv", "bp"):
            m[w] = full[w]
        in_maps.append(m)
    return in_maps


def kernel(**inputs):
    from concourse.bass_utils import run_bass_kernel_spmd

    nc = _get_graph()
    full = {
        k: np.ascontiguousarray(np.asarray(v, np.float32))
        for k, v in inputs.items()
    }
    res = run_bass_kernel_spmd(
        nc,
        make_in_maps(full),
        core_ids=list(range(NCORES)),
        trace=bool(os.environ.get("ATTN_TRACE")),
    )
    if res.exec_time_ns is not None:
        _graph_cache["exec_time_ns"] = res.exec_time_ns
        _graph_cache["profile_json"] = res.profile_json
        _graph_cache["trace"] = res.instructions_and_trace
    out = np.concatenate([res.results[c]["out"] for c in range(NCORES)], axis=0)
    return out
